# revision 39
# baseline (speedup 1.0000x reference)
"""Multi-head attention (B=2, S=2048, D=1024, H=16) on 8 trn2 NeuronCores.

Sharding: core c handles heads {2c, 2c+1} for BOTH batches (tensor parallel by
head). Token axis is flattened b-major: T = B*S = 4096.
 - Q/K/V projections computed per-core for its 2 heads (column-sharded weights,
   host-transposed to [D, .] so contraction sits on partitions).
 - Attention in transposed orientation: scoresT[j,i] tiles on PE (both heads
   in one 2-bank psum tile), ONE exp on ACT per j-step covering both heads
   (scale=1/8 folded in), causal masking via a DVE triangular-mask multiply,
   PV with ones-augmented V so the softmax denominator falls out of the
   same accumulation (row 64 of the PV psum). Normalization: the denom row is
   stream-transposed (32x32 DVE transpose) so the reciprocal runs on a
   free-dim-16 view (~60ns vs 3.3us on the 512-wide row), transposed back,
   broadcast across 64 partitions on gpsimd, DVE mul into CT.
 - Output projection: each core computes its 128-row (2-head) partial of
   out = concat @ Wo^T for ALL tokens; the 8 partials are summed on the host.
 - Streaming: xq/xk/xv are loaded in 512-token column slices (2KB DMA lines,
   paired) and Q/K/V projections are interleaved into the attention block
   loop, so the 24MB input stream hides behind attention compute.
All matmuls bf16 with fp32 PSUM accumulation. Host pre-transposes/casts inputs.
"""

import sys

sys.path.insert(0, "/opt/trn_rl_repo")

import numpy as np
import ml_dtypes

import concourse.bass as bass
import concourse.mybir as mybir
import concourse.tile as tile
from concourse import bacc
from concourse import bass_utils

B, S, D, H = 2, 2048, 1024, 16
DK = D // H              # 64
N_CORES = 8
HPC = H // N_CORES       # heads per core (2)
EPC = HPC * DK           # 128 projected cols per core
T = B * S                # 4096 flattened tokens
TOK = T // N_CORES       # 512 output tokens per core
IT = 512                 # i (query) tile
JT = 128                 # j (key) tile
NIT = S // IT            # 4 i-tiles per batch
NJT = S // JT            # 16 j-tiles per batch
NST = T // IT            # 8 projection token tiles
ND = D // 128            # 8 contraction tiles
VST = DK + 1             # 65: V block width with ones column

bf16 = mybir.dt.bfloat16
f32 = mybir.dt.float32
f16 = mybir.dt.float16
BF = ml_dtypes.bfloat16

_CACHE: dict = {}


def _store_junk(nc, tc, out):
    import concourse.mybir as _mb
    with tc.tile_pool(name="junk", bufs=1) as jp:
        jt_ = jp.tile([128, D], _mb.dt.float32, name="junk")
        nc.vector.memset(jt_[:], 0.0)
        for tt in range(TOK // 128):
            nc.sync.dma_start(out.ap()[128 * tt:128 * (tt + 1), :], jt_[:])


def _build(mode: str, repeats: int = 1, upto: str = "full"):
    """mode: 'causal' | 'none' | 'generic'. repeats>1 builds a timing variant
    that executes the whole body N times in one NEFF. upto: 'full' | 'p2' |
    'p1' truncates after attention / projections (timing ablation only)."""
    nc = bacc.Bacc("TRN2", target_bir_lowering=False, debug=False,
                   enable_asserts=False, num_devices=N_CORES)

    xq = nc.dram_tensor("xq", [D, T], bf16, kind="ExternalInput")
    xk = nc.dram_tensor("xk", [D, T], bf16, kind="ExternalInput")
    xv = nc.dram_tensor("xv", [D, T], bf16, kind="ExternalInput")
    wq = nc.dram_tensor("wq", [D, EPC], bf16, kind="ExternalInput")
    wk = nc.dram_tensor("wk", [D, EPC], bf16, kind="ExternalInput")
    wv = nc.dram_tensor("wv", [D, EPC], bf16, kind="ExternalInput")
    wo = nc.dram_tensor("wo", [128, D], bf16, kind="ExternalInput")
    if mode == "generic":
        bias = nc.dram_tensor("bias", [S, S], bf16, kind="ExternalInput")
    out = nc.dram_tensor("out", [T, D], f16, kind="ExternalOutput")

    Exp = mybir.ActivationFunctionType.Exp

    with tile.TileContext(nc) as tc:
      for _rep in range(repeats):
        with (
            tc.tile_pool(name="consts", bufs=1) as consts,
            tc.tile_pool(name="persist", bufs=1) as persist,
            tc.tile_pool(name="dram", bufs=1, space="DRAM") as dram,
        ):
            # --- persistent SBUF tensors ---
            wqb = consts.tile([128, ND * EPC], bf16, tag="wqb", name="wqb")
            wkb = consts.tile([128, ND * EPC], bf16, tag="wkb", name="wkb")
            wvb = consts.tile([128, ND * EPC], bf16, tag="wvb", name="wvb")
            wob = consts.tile([128, D], bf16, tag="wob", name="wob")
            wq_sb = [wqb[:, EPC * d:EPC * (d + 1)] for d in range(ND)]
            wk_sb = [wkb[:, EPC * d:EPC * (d + 1)] for d in range(ND)]
            wv_sb = [wvb[:, EPC * d:EPC * (d + 1)] for d in range(ND)]

            QT = persist.tile([128, T], bf16, tag="QT")
            KT = persist.tile([128, T], bf16, tag="KT")
            CT = persist.tile([128, T], bf16, tag="CT")
            NJ_ALL = T // JT     # 32 j-tiles across both batches
            V_all = persist.tile([128, NJ_ALL * HPC * VST], bf16, tag="V_all")
            v4 = V_all[:].rearrange("p (t h c) -> p (t h) c",
                                    t=NJ_ALL, h=HPC, c=VST)
            nc.vector.memset(v4[:, :, DK:DK + 1], 1.0)
            # lower-triangular bf16 mask for the diagonal score tiles
            # (applied as a DVE multiply -- keeps exp-gated work off the
            # in-order gpsimd queue, which also carries the tail broadcasts)
            tri = consts.tile([128, JT], bf16, tag="tri", name="tri")
            nc.vector.memset(tri[:], 1.0)
            nc.gpsimd.affine_select(
                out=tri[:], in_=tri[:],
                compare_op=mybir.AluOpType.is_ge,
                fill=0.0, base=0, pattern=[[1, JT]],
                channel_multiplier=-1)

            def _wload(wdram, wbig):
                nc.sync.dma_start(
                    wbig[:].rearrange("p (d e) -> p d e", d=ND, e=EPC),
                    wdram.ap().rearrange("(d p) e -> p d e", p=128))
            _wload(wq, wqb)
            _wload(wk, wkb)

            if upto == "p1":
                _store_junk(nc, tc, out)
                continue
            # Two heads interleaved per j-tile: breaks the pt->exp->po latency
            # chain and puts the two K=64 matmuls on different PE row groups
            # (base partitions 0/64) so they run concurrently on the array.
            with (
                tc.tile_pool(name="psP", bufs=1, space="PSUM") as psP,
                tc.tile_pool(name="psO", bufs=1, space="PSUM") as psO,
                tc.tile_pool(name="sbE", bufs=1) as sbE,
                tc.tile_pool(name="sbR", bufs=1) as sbR,
                tc.tile_pool(name="biasp", bufs=4) as biasp,
                tc.tile_pool(name="sbF", bufs=1) as sbF,
                tc.tile_pool(name="xvp", bufs=3) as xvp,
            ):
                final_q = []
                xsl = {}

                def _dma_slice(s, w=1):
                    # stage token slices [IT*s, IT*(s+w)) of xq/xk/xv into
                    # SBUF; compute is emitted later (_proj/_v_compute) once
                    # the transfer has had a block's worth of time to land.
                    # w=2 doubles the per-partition DMA line to 2KB.
                    ts = [[] for _ in range(w)]
                    for nm, dt_ in (("q", xq), ("k", xk), ("v", xv)):
                        t = xvp.tile([128, ND, w * IT], bf16, bufs=2,
                                     tag=f"x{nm}sl", name=f"x{nm}sl")
                        nc.sync.dma_start(
                            t[:],
                            dt_.ap()[:, IT * s:IT * (s + w)].rearrange(
                                "(d p) t -> p d t", p=128))
                        for k in range(w):
                            ts[k].append(t[:][:, :, IT * k:IT * (k + 1)])
                    for k in range(w):
                        xsl[s + k] = ts[k]

                def _proj_compute(s):
                    # Q/K projections for token slice s (column-sharded wq/wk
                    # stationary, 512-token moving dim)
                    tq, tk, _ = xsl[s]
                    for tx, wsb, dest in ((tq, wq_sb, QT), (tk, wk_sb, KT)):
                        psj = psP.tile([128, IT], f32, tag="pf", bufs=2,
                                       name="psj")
                        for d in range(ND):
                            nc.tensor.matmul(psj[:], wsb[d], tx[:, d, :],
                                             start=(d == 0),
                                             stop=(d == ND - 1))
                        nc.vector.tensor_copy(
                            dest[:, IT * s:IT * (s + 1)], psj[:])

                def _v_compute(s):
                    # V projection for slice s: out[j, e] per j-tile so PV can
                    # consume V_all blocks as soon as each j-tile lands.
                    xt = xsl[s][2]
                    for jl4 in range(IT // JT):
                        jt = (IT // JT) * s + jl4
                        psv = psP.tile([128, EPC], f32, tag="pf", bufs=2,
                                       name="psv")
                        for d in range(ND):
                            nc.tensor.matmul(
                                psv[:], xt[:, d, JT * jl4:JT * (jl4 + 1)],
                                wv_sb[d], start=(d == 0), stop=(d == ND - 1))
                        dst = V_all[:, VST * HPC * jt:VST * HPC * (jt + 1)]
                        nc.vector.tensor_copy(
                            dst.rearrange("p (h c) -> p h c",
                                          h=HPC, c=VST)[:, :, 0:DK],
                            psv[:].rearrange("p (h c) -> p h c",
                                             h=HPC, c=DK))

                def _emit_final(fi0, split=False):
                    of = sbF.tile([128, (IT // 128) * D], f16, tag="of",
                                  bufs=2, name="of")
                    for tt in range(IT // 128):
                        t0 = fi0 + 128 * tt
                        for eh in range(2):
                            pf = psP.tile([128, IT], f32, tag="pf",
                                          bufs=2, name="pf")
                            nc.tensor.matmul(
                                pf[:], CT[:, t0:t0 + 128],
                                wob[:, 512 * eh:512 * (eh + 1)],
                                start=True, stop=True)
                            ofd = of[:, D * tt + 512 * eh:
                                     D * tt + 512 * (eh + 1)]
                            if split and eh == 1:
                                # kernel epilogue: ACT is out of exp work, so
                                # splitting the casts halves the DVE-paced
                                # drain of the last output blocks
                                nc.scalar.copy(ofd, pf[:])
                            else:
                                nc.vector.tensor_copy(ofd, pf[:])
                        if split:
                            nc.gpsimd.dma_start(
                                out.ap()[t0:t0 + 128, :],
                                of[:, D * tt:D * (tt + 1)])
                    if not split:
                        nc.gpsimd.dma_start(
                            out.ap()[fi0:fi0 + IT, :].rearrange(
                                "(tt p) e -> p tt e", p=128),
                            of[:].rearrange("p (tt e) -> p tt e",
                                            tt=IT // 128, e=D))

                def _copy_pos(tpos):
                    # evacuate the PV psum to SBUF right after the drain so
                    # the po slots free in ~1.5us instead of after the whole
                    # normalize chain (the next block's first PV reuses one)
                    pcs = []
                    for hl in range(HPC):
                        pc = sbR.tile([DK + 32, IT], f32, tag="posc",
                                      bufs=4, name="posc")
                        nc.vector.tensor_copy(pc[:], tpos[hl][0:DK + 32, :])
                        pcs.append(pc)
                    return pcs

                def _emit_tail(ti0, pcs):
                    # normalize the finished PV block into CT. The denom row
                    # (row DK of pos) is reciprocal'd in a transposed view:
                    # DVE stream-transpose moves it into 16 stride-32 columns
                    # (free dim 16 -> cheap reciprocal, vs 6.4ns/elem on a
                    # 512-wide row), then a second stream-transpose puts the
                    # reciprocals back into row form for the broadcast.
                    for hl in range(HPC):
                        dn32 = sbR.tile([32, IT], f32, tag="dn32", bufs=2,
                                        name="dn32")
                        nc.vector.transpose(dn32[:], pcs[hl][DK:DK + 32, :])
                        dnv = dn32[:].rearrange("p (c k) -> p c k",
                                                c=IT // 32, k=32)[:, :, 0:1]
                        nc.vector.reciprocal(dnv, dnv)
                        rrow = sbR.tile([32, IT], f32, tag="rrow", bufs=2,
                                        name="rrow")
                        nc.vector.transpose(rrow[:], dn32[:])
                        pbs = sbR.tile([DK, IT], f32, tag="pbs", bufs=2,
                                       name="pbs")
                        nc.gpsimd.partition_broadcast(pbs[:], rrow[0:1, :])
                        nc.vector.tensor_mul(
                            CT[DK * hl:DK * (hl + 1), ti0:ti0 + IT],
                            pcs[hl][0:DK, :], pbs[:])

                if mode == "causal":
                    _dma_slice(0)
                    _wload(wv, wvb)
                    nc.sync.dma_start(wob[:], wo.ap())
                    _dma_slice(1)
                    _proj_compute(0)
                    _v_compute(0)
                    _dma_slice(2, w=2)
                    _proj_compute(1)
                    _v_compute(1)
                else:
                    _wload(wv, wvb)
                    nc.sync.dma_start(wob[:], wo.ap())
                    _dma_slice(0)
                    _dma_slice(1)
                    _proj_compute(0)
                    _v_compute(0)
                    _dma_slice(2, w=2)
                    _proj_compute(1)
                    _v_compute(1)
                    _dma_slice(4)
                    for s in (2, 3):
                        _proj_compute(s)
                        _v_compute(s)
                for sblk, (b, it) in enumerate(
                        (b, it) for b in range(B) for it in range(NIT)):
                        tok0 = S * b
                        il0 = IT * it            # batch-local i offset
                        i0 = tok0 + il0
                        njt = (il0 + IT) // JT if mode == "causal" else NJT
                        pos = [psO.tile([128, IT], f32, tag="po", bufs=2,
                                        name=f"po{hl}") for hl in range(HPC)]
                        pend = []   # (jl, hl, ex) exp'd tiles awaiting PV

                        def _emit_pv(pjl, pex, poff, pnl):
                            pjabs = NJT * b + pjl
                            for phl in range(HPC):
                                voff = VST * (HPC * pjabs + phl)
                                nc.tensor.matmul(pos[phl][0:VST, poff:IT],
                                                 V_all[:, voff:voff + VST],
                                                 pex[:, phl, 0:pnl],
                                                 start=(pjl == 0),
                                                 stop=(pjl == njt - 1))

                        for jl in range(njt):
                            if jl == 1:
                                snx = sblk + (4 if mode == "causal" else 5)
                                if snx < NST and snx not in xsl:
                                    _dma_slice(snx, w=min(2, NST - snx))
                            jabs = NJT * b + jl
                            j0 = JT * jl             # batch-local j offset
                            diag = mode == "causal" and j0 > il0 - 1
                            # live i-columns of this block: i >= j0 (causal)
                            off = max(0, j0 - il0) if mode == "causal" else 0
                            nl = IT - off            # live width
                            bs = None
                            if mode == "generic":
                                bs = biasp.tile([128, IT], bf16, tag="bias",
                                                name="bs")
                                nc.sync.dma_start(
                                    bs[:],
                                    bias.ap()[JT * jl:JT * (jl + 1),
                                              il0:il0 + IT])
                            # both heads' scores in one 2-bank psum tile;
                            # ONE exp covers them (amortizes the ~260ns ACT
                            # per-op overhead -- ACT paces the dense j-loops)
                            pt = psP.tile([128, HPC, IT], f32, tag="pt",
                                          bufs=2, name="pt")
                            for hl in range(HPC):
                                pb = 64 * hl
                                nc.tensor.matmul(
                                    pt[:, hl, 0:nl],
                                    KT[pb:pb + DK, JT * jabs:JT * (jabs + 1)],
                                    QT[pb:pb + DK, i0 + off:i0 + IT],
                                    start=True, stop=True)
                                if bs is not None:
                                    nc.vector.tensor_add(pt[:, hl, 0:nl],
                                                         pt[:, hl, 0:nl],
                                                         bs[:, off:IT])
                            ex = sbE.tile([128, HPC, IT], bf16, tag="expp",
                                          bufs=6, name="ex")
                            nc.scalar.activation(ex[:, :, 0:nl],
                                                 pt[:, :, 0:nl], Exp,
                                                 scale=0.125)
                            if diag:
                                # triangular part lives in the first JT
                                # live cols: keep iff j0+p <= j0+f
                                for hl in range(HPC):
                                    nc.vector.tensor_mul(
                                        ex[:, hl, 0:JT], ex[:, hl, 0:JT],
                                        tri[:])
                            pend.append((jl, ex, off, nl))
                            # emit PV one j-step behind so each po matmul's
                            # exp input was produced during the previous
                            # j-step's score matmuls (keeps PE from stalling)
                            while len(pend) > 2:
                                _emit_pv(*pend.pop(0))
                        for p5 in pend:
                            _emit_pv(*p5)
                        # normalize/transpose this block's PV into CT, then
                        # the output projection for the PREVIOUS i-block
                        # (deferred so its chain overlaps this block's work)
                        pcs = _copy_pos(pos)
                        snc = sblk + (2 if mode == "causal" else 4)
                        if snc < NST:
                            _proj_compute(snc)
                            _v_compute(snc)
                        final_q.append(i0)
                        if sblk == NST - 1:
                            # last block: CT must be ready before the final
                            # projections, so the tail goes first
                            _emit_tail(i0, pcs)
                        if len(final_q) > 1:
                            _emit_final(final_q.pop(0), split=(sblk == NST - 1))
                        if sblk < NST - 1:
                            # tail last: its DVE chain (with gpsimd round
                            # trips) queues BEHIND the of-casts that free pf
                            # slots; nothing needs CT(n) until the next
                            # block's _emit_final
                            _emit_tail(i0, pcs)

                for fi0 in final_q:
                    _emit_final(fi0, split=True)

    nc.compile()
    return nc


def _prep(inputs, mode):
    query = np.asarray(inputs["query"], np.float32)
    key = np.asarray(inputs["key"], np.float32)
    value = np.asarray(inputs["value"], np.float32)
    Wq = np.asarray(inputs["Wq"], np.float32)
    Wk = np.asarray(inputs["Wk"], np.float32)
    Wv = np.asarray(inputs["Wv"], np.float32)
    Wo = np.asarray(inputs["Wo"], np.float32)

    xqT = np.ascontiguousarray(query.reshape(T, D).T).astype(BF)
    xkT = np.ascontiguousarray(key.reshape(T, D).T).astype(BF)
    xvT = np.ascontiguousarray(value.reshape(T, D).T).astype(BF)
    woT = np.ascontiguousarray(Wo.T).astype(BF)
    woT_loc = [np.ascontiguousarray(woT[128 * c:128 * (c + 1), :])
               for c in range(N_CORES)]
    wqT = [np.ascontiguousarray(Wq[EPC * c:EPC * (c + 1), :].T).astype(BF)
           for c in range(N_CORES)]
    wkT = [np.ascontiguousarray(Wk[EPC * c:EPC * (c + 1), :].T).astype(BF)
           for c in range(N_CORES)]
    wvT = [np.ascontiguousarray(Wv[EPC * c:EPC * (c + 1), :].T).astype(BF)
           for c in range(N_CORES)]

    biasT = None
    if mode == "generic":
        m2 = np.asarray(inputs["mask"])[0, 0]
        biasT = np.ascontiguousarray(
            np.where(m2.T == 0, np.float32(-1e9), np.float32(0.0))).astype(BF)

    in_maps = []
    for c in range(N_CORES):
        m = {"xq": xqT, "xk": xkT, "xv": xvT,
             "wq": wqT[c], "wk": wkT[c], "wv": wvT[c], "wo": woT_loc[c]}
        if biasT is not None:
            m["bias"] = biasT
        in_maps.append(m)
    return in_maps


def _mask_mode(mask):
    m2 = np.asarray(mask)[0, 0]
    if (m2 == 1).all():
        return "none"
    if np.array_equal(m2 != 0, np.tril(np.ones(m2.shape, dtype=bool))):
        return "causal"
    return "generic"


def kernel(**inputs) -> np.ndarray:
    mode = _mask_mode(inputs["mask"])
    if mode not in _CACHE:
        _CACHE[mode] = _build(mode)
    nc = _CACHE[mode]
    in_maps = _prep(inputs, mode)
    res = bass_utils.run_bass_kernel_spmd(nc, in_maps,
                                          core_ids=list(range(N_CORES)))
    out = res.results[0]["out"].astype(np.float32)
    for c in range(1, N_CORES):
        out += res.results[c]["out"]
    return out.reshape(B, S, D)


if __name__ == "__main__":
    rng = np.random.default_rng(0)
    inputs = {
        "query": rng.standard_normal((B, S, D)).astype(np.float32),
        "key": rng.standard_normal((B, S, D)).astype(np.float32),
        "value": rng.standard_normal((B, S, D)).astype(np.float32),
        "mask": np.tril(np.ones((S, S), np.int32))[None, None],
        "Wq": (rng.standard_normal((D, D)) / 32).astype(np.float32),
        "Wk": (rng.standard_normal((D, D)) / 32).astype(np.float32),
        "Wv": (rng.standard_normal((D, D)) / 32).astype(np.float32),
        "Wo": (rng.standard_normal((D, D)) / 32).astype(np.float32),
    }
    got = kernel(**inputs)
    print("kernel ran, out shape", got.shape, "finite:", np.isfinite(got).all())

